# revision 12
# baseline (speedup 1.0000x reference)
"""Trainium2 Bass kernel for an attention block (B=8, H=W=32, C=256, 4 heads).

Sharding: data-parallel over batch — one batch element per NeuronCore (8 cores).
Each core computes, for its x_b [N=1024, C=256]:
    qkv = x @ W_qkv ; per-head attention ; out-proj + bias + residual.

fp8 (e4m3) DoubleRow matmuls everywhere — every matmul contracts K=256 per
instruction (2 fp8 weights/cell), halving PE streaming vs bf16. Scale plan
(power-of-2 host-side scaling keeps every fp8 tensor in range):
  wq_hw = 16*Wq, wk_hw = 16*Wk  ->  S_hw = 4096 * S_true  (S_true = q.k/16)
  exp via ScalarE activation: expS = exp(S_hw/4096 - 3.0)  -> fp8 (max ~ e^4.5)
  wv_hw = 16*Wv -> v_hw = 16*v ; denominator matmul uses a 2.0-constant lhsT
  -> rbc = 1/(2*D) ; ocT = (16/2)*O_true = 8*O_true (fp8)
  wo_hw = 4*Wo -> psum = 32*(out-proj) ; evacuation res = psum/32 + x (DVE).

Startup: input DMAs split across the 3 HW DMA queues (x first, then fp8
weights); ~20 dummy bf16 matmuls on zeros warm the PE HAM clock gate
(1.2->2.4 GHz) during the DMA/engine-boot window; x is transposed on-chip in
bf16 and cast to fp8.

Schedule: software-pipelined at tile granularity. Each head's S^T matmuls
(exp on ScalarE is the per-head critical path, ~1.34us/tile) are interleaved
with "items" — v projection, next head's q/k projection, previous head's
denominator/O chunks — so the PE never waits on exp. The tail interleaves
the last head's O chunks with the out-projection's i-tiles.
"""

import numpy as np
import ml_dtypes

import concourse.bass as bass
import concourse.tile as tile
from concourse import bacc, mybir
from concourse import bass_utils
from concourse.masks import make_identity

P = 128
N_TOK = 1024          # tokens per batch element (32*32)
C = 256               # channels
NH = 4                # heads
HD = 256              # head dim (= C)
NT = N_TOK // P       # 8 token tiles
F32 = mybir.dt.float32
F32R = mybir.dt.float32r
BF16 = mybir.dt.bfloat16
FP8 = mybir.dt.float8e4
DR = mybir.MatmulPerfMode.DoubleRow
EXP_SHIFT = 3.0       # softmax logit shift; S_true in [-6.8, 7.5] for this data
EXP_SCALE = 1.0 / 4096.0
N_WARM = 9            # dummy matmuls to warm the HAM clock gate during DMA-in


def _build_program(with_bias):
    nc = bacc.Bacc("TRN2", target_bir_lowering=False, debug=False, num_devices=8)

    x_d = nc.dram_tensor("x", [N_TOK, C], BF16, kind="ExternalInput").ap()
    wq_d = nc.dram_tensor("wq", [C, NH * HD], FP8, kind="ExternalInput").ap()
    wk_d = nc.dram_tensor("wk", [C, NH * HD], FP8, kind="ExternalInput").ap()
    wv_d = nc.dram_tensor("wv", [C, NH * HD], FP8, kind="ExternalInput").ap()
    wo_d = nc.dram_tensor("wo", [NH * HD, C], FP8, kind="ExternalInput").ap()
    bo_d = nc.dram_tensor("bo", [1, C], F32R, kind="ExternalInput").ap()
    y_d = nc.dram_tensor("y", [N_TOK, C], F32, kind="ExternalOutput").ap()
    y_r = y_d.rearrange("(t p) c -> p t c", p=P)

    with tile.TileContext(nc) as tc:
        with (
            tc.tile_pool(name="singles", bufs=1) as singles,
            tc.tile_pool(name="expp", bufs=2) as expp,
            tc.tile_pool(name="respool", bufs=6) as respool,
            tc.tile_pool(name="ps_s", bufs=2, space="PSUM") as ps_s,
            tc.tile_pool(name="ps_512", bufs=3, space="PSUM") as ps_512,
        ):
            # ---- constants; warm tiles first so the PE warmup starts ASAP ------
            warmL = singles.tile([P, P], BF16)
            nc.gpsimd.memset(warmL[:], 0.0)
            warmR = singles.tile([P, 512], BF16)
            nc.gpsimd.memset(warmR[:], 0.0)
            id_bf = singles.tile([P, P], BF16)
            make_identity(nc, id_bf[:])
            twos8 = singles.tile([P, 2, P], FP8)
            nc.vector.memset(twos8[:], 2.0)
            ebias = singles.tile([P, 1], F32)
            nc.vector.memset(ebias[:], -EXP_SHIFT)
            ones_f = singles.tile([1, P], F32)
            nc.vector.memset(ones_f[:], 1.0)
            ones_sb = singles.tile([1, P], F32R)
            nc.vector.tensor_copy(ones_sb[:], ones_f[:])

            # ---- input DMAs: x chunks first, then fp8 weights, on the 3 HW
            # DMA-capable queues (sync/scalar/gpsimd) ---------------------------
            x_r = x_d.rearrange("(t p) c -> p t c", p=P)
            xbf = singles.tile([P, NT, C], BF16)
            for t in range(3):
                nc.sync.dma_start(xbf[:, t:t + 1, :], x_r[:, t:t + 1, :])
            for t in range(3, 6):
                nc.scalar.dma_start(xbf[:, t:t + 1, :], x_r[:, t:t + 1, :])
            for t in range(6, 8):
                nc.gpsimd.dma_start(xbf[:, t:t + 1, :], x_r[:, t:t + 1, :])
            wq_sb = singles.tile([P, 2, NH * HD], FP8)
            nc.sync.dma_start(wq_sb[:], wq_d.rearrange("(s p) d -> p s d", p=P))
            wk_sb = singles.tile([P, 2, NH * HD], FP8)
            nc.scalar.dma_start(wk_sb[:], wk_d.rearrange("(s p) d -> p s d", p=P))
            wv_sb = singles.tile([P, 2, NH * HD], FP8)
            nc.gpsimd.dma_start(wv_sb[:], wv_d.rearrange("(s p) d -> p s d", p=P))
            wo_sb = singles.tile([P, NT, C], FP8)
            nc.gpsimd.dma_start(wo_sb[:], wo_d.rearrange("(k p) c -> p k c", p=P))
            bo_sb = singles.tile([1, C], F32R)
            if with_bias:
                nc.gpsimd.dma_start(bo_sb[:], bo_d[:])

            # ---- HAM warmup: dummy bf16 matmuls on zeros while DMAs stream -----
            for i in range(N_WARM):
                pw = ps_512.tile([P, 512], F32, tag="ps512")
                nc.tensor.matmul(pw[:], warmL[:], warmR[:], start=True, stop=True)

            # ---- transpose x (PE, chasing DMA tiles), cast to fp8 x^T ----------
            xT8 = singles.tile([P, 2, N_TOK], FP8)    # [c-part, c-sub, i]
            pts = [
                ps_512.tile([P, N_TOK], BF16, tag="ps512", name=f"pt{cc}")
                for cc in range(2)
            ]
            for t in (0, 3, 6, 1, 4, 7, 2, 5):   # x-tile DMA arrival order
                for cc in range(2):
                    nc.tensor.transpose(
                        pts[cc][:, t * P:(t + 1) * P],
                        xbf[:, t, cc * P:(cc + 1) * P],
                        id_bf[:],
                    )
            nc.vector.tensor_copy(xT8[:, 0, :], pts[0][:])
            nc.scalar.copy(xT8[:, 1, :], pts[1][:])

            # ---- phase 1 helpers (all fp8 DoubleRow, K=256 per matmul) ---------
            qTa = singles.tile([P, 2 * NH, N_TOK], FP8)   # [d-part, h*2+dsub, i]
            kTa = singles.tile([P, 2 * NH, N_TOK], FP8)
            va = singles.tile([P, NT, NH * HD], FP8)      # [i-part, t, h*HD+d]

            def qk_half(w_sb, dstT, dc, ih, on_scalar=False):
                pq = ps_512.tile([P, 512], F32, tag="ps512")
                nc.tensor.matmul(
                    pq[:],
                    w_sb[:, :, dc * P:(dc + 1) * P],
                    xT8[:, :, ih * 512:(ih + 1) * 512],
                    start=True, stop=True, perf_mode=DR,
                )
                dst = dstT[:, dc, ih * 512:(ih + 1) * 512]
                if on_scalar:
                    nc.scalar.copy(dst, pq[:])
                else:
                    nc.vector.tensor_copy(dst, pq[:])

            def qk_chunk(w_sb, dstT, dc, on_scalar=False):
                for ih in range(2):
                    qk_half(w_sb, dstT, dc, ih, on_scalar=on_scalar)

            def v_chunk(ic):
                for dh in range(2):
                    pv = ps_512.tile([P, 512], F32, tag="ps512")
                    nc.tensor.matmul(
                        pv[:],
                        xT8[:, :, ic * P:(ic + 1) * P],
                        wv_sb[:, :, dh * 512:(dh + 1) * 512],
                        start=True, stop=True, perf_mode=DR,
                    )
                    nc.vector.tensor_copy(va[:, ic, dh * 512:(dh + 1) * 512], pv[:])

            # ---- phase 2 helpers ----------------------------------------------
            ocT = singles.tile([P, NT, N_TOK], FP8)   # [d-part, hC-chunk, i] = 8*O^T

            def den_chunk(h, expSt, rbc, ih):
                pd = ps_512.tile([P, 512], F32, tag="ps512")
                for jp in range(4):
                    nc.tensor.matmul(
                        pd[:],
                        twos8[:],
                        expSt[:, 2 * jp:2 * jp + 2, ih * 512:(ih + 1) * 512],
                        start=(jp == 0), stop=(jp == 3), perf_mode=DR,
                    )
                nc.vector.reciprocal_approx_fast(
                    rbc[:, ih * 512:(ih + 1) * 512], pd[:]
                )

            def o_chunk(h, expSt, rbc, dt_, ih):
                d0 = (2 * h + dt_) * P
                po = ps_512.tile([P, 512], F32, tag="ps512")
                for jp in range(4):
                    nc.tensor.matmul(
                        po[:],
                        va[:, 2 * jp:2 * jp + 2, d0:d0 + P],
                        expSt[:, 2 * jp:2 * jp + 2, ih * 512:(ih + 1) * 512],
                        start=(jp == 0), stop=(jp == 3), perf_mode=DR,
                    )
                nc.vector.tensor_mul(
                    ocT[:, 2 * h + dt_, ih * 512:(ih + 1) * 512],
                    po[:],
                    rbc[:, ih * 512:(ih + 1) * 512],
                )

            def do_items(h, expSt):
                rbc = expp.tile([P, N_TOK], F32, tag="rbc")
                its = [lambda ih=ih: den_chunk(h, expSt, rbc, ih) for ih in range(2)]
                its += [
                    lambda dt_=dt_, ih=ih: o_chunk(h, expSt, rbc, dt_, ih)
                    for ih in range(2) for dt_ in range(2)
                ]
                return its

            def s_head(h, items):
                """S^T matmuls + exp, with `items` interleaved between j-tiles."""
                expSt = expp.tile([P, NT, N_TOK], FP8, tag="expS")
                done = 0
                for jt in range(NT):
                    pss = ps_s.tile([P, N_TOK], F32, tag="psS")
                    for ih in range(2):
                        nc.tensor.matmul(
                            pss[:, ih * 512:(ih + 1) * 512],
                            kTa[:, 2 * h:2 * h + 2, jt * P:(jt + 1) * P],
                            qTa[:, 2 * h:2 * h + 2, ih * 512:(ih + 1) * 512],
                            start=True, stop=True, perf_mode=DR,
                        )
                    nc.scalar.activation(
                        expSt[:, jt, :], pss[:],
                        mybir.ActivationFunctionType.Exp,
                        bias=ebias[:], scale=EXP_SCALE,
                    )
                    want = (len(items) * (jt + 1) + NT - 1) // NT
                    while done < min(want, len(items)):
                        items[done]()
                        done += 1
                while done < len(items):
                    items[done]()
                    done += 1
                return expSt

            def ph3_tile(it):
                pr = ps_512.tile([P, 512], F32, tag="ps512")
                out = pr[:, :C]
                if with_bias:
                    nc.tensor.matmul(
                        out, ones_sb[:], bo_sb[:], start=True, stop=False,
                    )
                for kp in range(4):
                    nc.tensor.matmul(
                        out,
                        ocT[:, 2 * kp:2 * kp + 2, it * P:(it + 1) * P],
                        wo_sb[:, 2 * kp:2 * kp + 2, :],
                        start=(kp == 0 and not with_bias), stop=(kp == 3),
                        perf_mode=DR,
                    )
                res = respool.tile([P, C], F32, tag="res")
                nc.vector.scalar_tensor_tensor(
                    res[:], out, 1.0 / 32.0, xbf[:, it, :],
                    op0=mybir.AluOpType.mult, op1=mybir.AluOpType.add,
                )
                eng = nc.sync if it % 2 == 0 else nc.scalar
                eng.dma_start(y_r[:, it, :], res[:])

            # ---- the pipeline --------------------------------------------------
            for ih in range(2):
                qk_half(wq_sb, qTa, 0, ih)
                qk_half(wk_sb, kTa, 0, ih, on_scalar=True)
                qk_half(wq_sb, qTa, 1, ih)
                qk_half(wk_sb, kTa, 1, ih, on_scalar=True)

            exp_tiles = [None] * NH
            items0 = [lambda ic=ic: v_chunk(ic) for ic in range(NT)]
            for dc in range(2, 4):
                items0.append(lambda dc=dc: qk_chunk(wq_sb, qTa, dc))
                items0.append(lambda dc=dc: qk_chunk(wk_sb, kTa, dc))
            exp_tiles[0] = s_head(0, items0)

            for h in range(1, NH):
                items = do_items(h - 1, exp_tiles[h - 1])
                if h + 1 < NH:
                    for dc in range(2 * (h + 1), 2 * (h + 1) + 2):
                        items.append(lambda dc=dc: qk_chunk(wq_sb, qTa, dc))
                        items.append(lambda dc=dc: qk_chunk(wk_sb, kTa, dc))
                exp_tiles[h] = s_head(h, items)

            # ---- tail: last head's denom/O interleaved with the out-proj ------
            rbc3 = expp.tile([P, N_TOK], F32, tag="rbc")
            expSt3 = exp_tiles[NH - 1]
            for ih in range(2):
                den_chunk(NH - 1, expSt3, rbc3, ih)
            o_chunk(NH - 1, expSt3, rbc3, 0, 0)
            o_chunk(NH - 1, expSt3, rbc3, 1, 0)
            for it in range(4):
                ph3_tile(it)
                if it < 2:
                    o_chunk(NH - 1, expSt3, rbc3, it, 1)
            for it in range(4, NT):
                ph3_tile(it)

    nc.compile()
    return nc


_NC_CACHE = {}


def _get_program(with_bias):
    key = ("nc", with_bias)
    if key not in _NC_CACHE:
        _NC_CACHE[key] = _build_program(with_bias)
    return _NC_CACHE[key]


def _fp8(a):
    return np.clip(a, -240.0, 240.0).astype(ml_dtypes.float8_e4m3fn)


def _make_in_maps(x, W_qkv, W_out, b_out):
    B = x.shape[0]
    x = np.ascontiguousarray(x.reshape(B, N_TOK, C), dtype=np.float32).astype(
        ml_dtypes.bfloat16
    )
    # W_qkv [C, h*3C]: head-major columns; q slot < C, k slot < 2C, v rest.
    w = np.asarray(W_qkv, dtype=np.float32).reshape(C, NH, 3 * C)
    wq = _fp8(np.ascontiguousarray(w[:, :, :C].reshape(C, NH * HD)) * 16.0)
    wk = _fp8(np.ascontiguousarray(w[:, :, C:2 * C].reshape(C, NH * HD)) * 16.0)
    wv = _fp8(np.ascontiguousarray(w[:, :, 2 * C:].reshape(C, NH * HD)) * 16.0)
    wo = _fp8(np.asarray(W_out, dtype=np.float32) * 4.0)
    bo = np.ascontiguousarray(
        np.asarray(b_out, dtype=np.float32).reshape(1, C) * 32.0
    )
    return [
        {"x": x[b], "wq": wq, "wk": wk, "wv": wv, "wo": wo, "bo": bo}
        for b in range(B)
    ]


def run_spmd(x, W_qkv, W_out, b_out, **runner_kwargs):
    """Run on the 8 cores; returns (BassKernelResults, assembled output)."""
    with_bias = bool(np.any(np.asarray(b_out)))
    nc = _get_program(with_bias)
    in_maps = _make_in_maps(x, W_qkv, W_out, b_out)
    res = bass_utils.run_bass_kernel_spmd(
        nc, in_maps, core_ids=list(range(8)), **runner_kwargs
    )
    B, H, W = x.shape[0], x.shape[1], x.shape[2]
    y = np.stack([res.results[b]["y"] for b in range(B)])
    return res, y.reshape(B, H, W, C).astype(np.float32)


def kernel(x, W_qkv, W_out, b_out):
    _, y = run_spmd(x, W_qkv, W_out, b_out)
    return y


# revision 13
# speedup vs baseline: 1.0297x; 1.0297x over previous
"""Trainium2 Bass kernel for an attention block (B=8, H=W=32, C=256, 4 heads).

Sharding: data-parallel over batch — one batch element per NeuronCore (8 cores).
Each core computes, for its x_b [N=1024, C=256]:
    qkv = x @ W_qkv ; per-head attention ; out-proj + bias + residual.

fp8 (e4m3) DoubleRow matmuls everywhere — every matmul contracts K=256 per
instruction (2 fp8 weights/cell), halving PE streaming vs bf16. Scale plan
(power-of-2 host-side scaling keeps every fp8 tensor in range):
  wq_hw = 16*Wq, wk_hw = 16*Wk  ->  S_hw = 4096 * S_true  (S_true = q.k/16)
  exp via ScalarE activation: expS = exp(S_hw/4096 - 3.0)  -> fp8 (max ~ e^4.5)
  wv_hw = 16*Wv -> v_hw = 16*v ; denominator matmul uses a 2.0-constant lhsT
  -> rbc = 1/(2*D) ; ocT = (16/2)*O_true = 8*O_true (fp8)
  wo_hw = 4*Wo -> psum = 32*(out-proj) ; evacuation res = psum/32 + x (DVE).

Startup: input DMAs split across the 3 HW DMA queues (x first, then fp8
weights); ~20 dummy bf16 matmuls on zeros warm the PE HAM clock gate
(1.2->2.4 GHz) during the DMA/engine-boot window; x is transposed on-chip in
bf16 and cast to fp8.

Schedule: software-pipelined at tile granularity. Each head's S^T matmuls
(exp on ScalarE is the per-head critical path, ~1.34us/tile) are interleaved
with "items" — v projection, next head's q/k projection, previous head's
denominator/O chunks — so the PE never waits on exp. The tail interleaves
the last head's O chunks with the out-projection's i-tiles.
"""

import numpy as np
import ml_dtypes

import concourse.bass as bass
import concourse.tile as tile
from concourse import bacc, mybir
from concourse import bass_utils
from concourse.masks import make_identity

P = 128
N_TOK = 1024          # tokens per batch element (32*32)
C = 256               # channels
NH = 4                # heads
HD = 256              # head dim (= C)
NT = N_TOK // P       # 8 token tiles
F32 = mybir.dt.float32
F32R = mybir.dt.float32r
BF16 = mybir.dt.bfloat16
FP8 = mybir.dt.float8e4
DR = mybir.MatmulPerfMode.DoubleRow
EXP_SHIFT = 3.0       # softmax logit shift; S_true in [-6.8, 7.5] for this data
EXP_SCALE = 1.0 / 4096.0
N_WARM = 9            # dummy matmuls to warm the HAM clock gate during DMA-in


def _build_program(with_bias):
    nc = bacc.Bacc("TRN2", target_bir_lowering=False, debug=False, num_devices=8)

    x_d = nc.dram_tensor("x", [N_TOK, C], BF16, kind="ExternalInput").ap()
    wq_d = nc.dram_tensor("wq", [C, NH * HD], FP8, kind="ExternalInput").ap()
    wk_d = nc.dram_tensor("wk", [C, NH * HD], FP8, kind="ExternalInput").ap()
    wv_d = nc.dram_tensor("wv", [C, NH * HD], FP8, kind="ExternalInput").ap()
    wo_d = nc.dram_tensor("wo", [NH * HD, C], FP8, kind="ExternalInput").ap()
    bo_d = nc.dram_tensor("bo", [1, C], F32R, kind="ExternalInput").ap()
    y_d = nc.dram_tensor("y", [N_TOK, C], F32, kind="ExternalOutput").ap()
    y_r = y_d.rearrange("(t p) c -> p t c", p=P)

    with tile.TileContext(nc) as tc:
        with (
            tc.tile_pool(name="singles", bufs=1) as singles,
            tc.tile_pool(name="expp", bufs=2) as expp,
            tc.tile_pool(name="respool", bufs=6) as respool,
            tc.tile_pool(name="ps_s", bufs=2, space="PSUM") as ps_s,
            tc.tile_pool(name="ps_512", bufs=3, space="PSUM") as ps_512,
        ):
            # ---- constants; warm tiles first so the PE warmup starts ASAP ------
            warmL = singles.tile([P, P], BF16)
            nc.gpsimd.memset(warmL[:], 0.0)
            warmR = singles.tile([P, 512], BF16)
            nc.gpsimd.memset(warmR[:], 0.0)
            id_bf = singles.tile([P, P], BF16)
            make_identity(nc, id_bf[:])
            twos8 = singles.tile([P, 2, P], FP8)
            nc.vector.memset(twos8[:], 2.0)
            ebias = singles.tile([P, 1], F32)
            nc.vector.memset(ebias[:], -EXP_SHIFT)
            ones_f = singles.tile([1, P], F32)
            nc.vector.memset(ones_f[:], 1.0)
            ones_sb = singles.tile([1, P], F32R)
            nc.vector.tensor_copy(ones_sb[:], ones_f[:])

            # ---- input DMAs: x chunks first, then fp8 weights, on the 3 HW
            # DMA-capable queues (sync/scalar/gpsimd) ---------------------------
            x_r = x_d.rearrange("(t p) c -> p t c", p=P)
            xbf = singles.tile([P, NT, C], BF16)
            for t in range(3):
                nc.sync.dma_start(xbf[:, t:t + 1, :], x_r[:, t:t + 1, :])
            for t in range(3, 6):
                nc.scalar.dma_start(xbf[:, t:t + 1, :], x_r[:, t:t + 1, :])
            for t in range(6, 8):
                nc.gpsimd.dma_start(xbf[:, t:t + 1, :], x_r[:, t:t + 1, :])
            wq_sb = singles.tile([P, 2, NH * HD], FP8)
            nc.sync.dma_start(wq_sb[:], wq_d.rearrange("(s p) d -> p s d", p=P))
            wk_sb = singles.tile([P, 2, NH * HD], FP8)
            nc.scalar.dma_start(wk_sb[:], wk_d.rearrange("(s p) d -> p s d", p=P))
            wv_sb = singles.tile([P, 2, NH * HD], FP8)
            nc.gpsimd.dma_start(wv_sb[:], wv_d.rearrange("(s p) d -> p s d", p=P))
            wo_sb = singles.tile([P, NT, C], FP8)
            nc.gpsimd.dma_start(wo_sb[:], wo_d.rearrange("(k p) c -> p k c", p=P))
            bo_sb = singles.tile([1, C], F32R)
            if with_bias:
                nc.gpsimd.dma_start(bo_sb[:], bo_d[:])

            # ---- HAM warmup: dummy bf16 matmuls on zeros while DMAs stream -----
            for i in range(N_WARM):
                pw = ps_512.tile([P, 512], F32, tag="ps512")
                nc.tensor.matmul(pw[:], warmL[:], warmR[:], start=True, stop=True)

            # ---- transpose x (PE, chasing DMA tiles), cast to fp8 x^T ----------
            xT8 = singles.tile([P, 2, N_TOK], FP8)    # [c-part, c-sub, i]
            pts = [
                ps_512.tile([P, N_TOK], BF16, tag="ps512", name=f"pt{cc}")
                for cc in range(2)
            ]
            for t in (0, 3, 6, 1, 4, 7, 2, 5):   # x-tile DMA arrival order
                for cc in range(2):
                    nc.tensor.transpose(
                        pts[cc][:, t * P:(t + 1) * P],
                        xbf[:, t, cc * P:(cc + 1) * P],
                        id_bf[:],
                    )
            nc.vector.tensor_copy(xT8[:, 0, :], pts[0][:])
            nc.scalar.copy(xT8[:, 1, :], pts[1][:])

            # ---- phase 1 helpers (all fp8 DoubleRow, K=256 per matmul) ---------
            qTa = singles.tile([P, 2 * NH, N_TOK], FP8)   # [d-part, h*2+dsub, i]
            kTa = singles.tile([P, 2 * NH, N_TOK], FP8)
            va = singles.tile([P, NT, NH * HD], FP8)      # [i-part, t, h*HD+d]

            def qk_half(w_sb, dstT, dc, ih, on_scalar=False):
                pq = ps_512.tile([P, 512], F32, tag="ps512")
                nc.tensor.matmul(
                    pq[:],
                    w_sb[:, :, dc * P:(dc + 1) * P],
                    xT8[:, :, ih * 512:(ih + 1) * 512],
                    start=True, stop=True, perf_mode=DR,
                )
                dst = dstT[:, dc, ih * 512:(ih + 1) * 512]
                if on_scalar:
                    nc.scalar.copy(dst, pq[:])
                else:
                    nc.vector.tensor_copy(dst, pq[:])

            def qk_chunk(w_sb, dstT, dc, on_scalar=False):
                for ih in range(2):
                    qk_half(w_sb, dstT, dc, ih, on_scalar=on_scalar)

            def v_chunk(ic):
                for dh in range(2):
                    pv = ps_512.tile([P, 512], F32, tag="ps512")
                    nc.tensor.matmul(
                        pv[:],
                        xT8[:, :, ic * P:(ic + 1) * P],
                        wv_sb[:, :, dh * 512:(dh + 1) * 512],
                        start=True, stop=True, perf_mode=DR,
                    )
                    nc.vector.tensor_copy(va[:, ic, dh * 512:(dh + 1) * 512], pv[:])

            # ---- phase 2 helpers ----------------------------------------------
            ocT = singles.tile([P, NT, N_TOK], FP8)   # [d-part, hC-chunk, i] = 8*O^T

            def den_chunk(h, expSt, rbc, ih):
                pd = ps_512.tile([P, 512], F32, tag="ps512")
                for jp in range(4):
                    nc.tensor.matmul(
                        pd[:],
                        twos8[:],
                        expSt[:, 2 * jp:2 * jp + 2, ih * 512:(ih + 1) * 512],
                        start=(jp == 0), stop=(jp == 3), perf_mode=DR,
                    )
                nc.vector.reciprocal_approx_fast(
                    rbc[:, ih * 512:(ih + 1) * 512], pd[:]
                )

            def o_chunk(h, expSt, rbc, dt_, ih):
                d0 = (2 * h + dt_) * P
                po = ps_512.tile([P, 512], F32, tag="ps512")
                for jp in range(4):
                    nc.tensor.matmul(
                        po[:],
                        va[:, 2 * jp:2 * jp + 2, d0:d0 + P],
                        expSt[:, 2 * jp:2 * jp + 2, ih * 512:(ih + 1) * 512],
                        start=(jp == 0), stop=(jp == 3), perf_mode=DR,
                    )
                nc.vector.tensor_mul(
                    ocT[:, 2 * h + dt_, ih * 512:(ih + 1) * 512],
                    po[:],
                    rbc[:, ih * 512:(ih + 1) * 512],
                )

            def do_items(h, expSt):
                rbc = expp.tile([P, N_TOK], F32, tag="rbc")
                its = [lambda ih=ih: den_chunk(h, expSt, rbc, ih) for ih in range(2)]
                its += [
                    lambda dt_=dt_, ih=ih: o_chunk(h, expSt, rbc, dt_, ih)
                    for ih in range(2) for dt_ in range(2)
                ]
                return its

            def s_head(h, items):
                """S^T matmuls + exp, with `items` interleaved between j-tiles."""
                expSt = expp.tile([P, NT, N_TOK], FP8, tag="expS")
                done = 0
                for jt in range(NT):
                    pss = ps_s.tile([P, N_TOK], F32, tag="psS")
                    for ih in range(2):
                        nc.tensor.matmul(
                            pss[:, ih * 512:(ih + 1) * 512],
                            kTa[:, 2 * h:2 * h + 2, jt * P:(jt + 1) * P],
                            qTa[:, 2 * h:2 * h + 2, ih * 512:(ih + 1) * 512],
                            start=True, stop=True, perf_mode=DR,
                        )
                    nc.scalar.activation(
                        expSt[:, jt, :], pss[:],
                        mybir.ActivationFunctionType.Exp,
                        bias=ebias[:], scale=EXP_SCALE,
                    )
                    want = (len(items) * (jt + 1) + NT - 1) // NT
                    while done < min(want, len(items)):
                        items[done]()
                        done += 1
                while done < len(items):
                    items[done]()
                    done += 1
                return expSt

            def ph3_tile(it):
                pr = ps_s.tile([P, N_TOK], F32, tag="psS")
                out = pr[:, 512 * (it % 2):512 * (it % 2) + C]
                if with_bias:
                    nc.tensor.matmul(
                        out, ones_sb[:], bo_sb[:], start=True, stop=False,
                    )
                for kp in range(4):
                    nc.tensor.matmul(
                        out,
                        ocT[:, 2 * kp:2 * kp + 2, it * P:(it + 1) * P],
                        wo_sb[:, 2 * kp:2 * kp + 2, :],
                        start=(kp == 0 and not with_bias), stop=(kp == 3),
                        perf_mode=DR,
                    )
                res = respool.tile([P, C], F32, tag="res")
                nc.vector.scalar_tensor_tensor(
                    res[:], out, 1.0 / 32.0, xbf[:, it, :],
                    op0=mybir.AluOpType.mult, op1=mybir.AluOpType.add,
                )
                eng = nc.sync if it % 2 == 0 else nc.scalar
                eng.dma_start(y_r[:, it, :], res[:])

            # ---- the pipeline --------------------------------------------------
            for ih in range(2):
                qk_half(wq_sb, qTa, 0, ih)
                qk_half(wk_sb, kTa, 0, ih, on_scalar=True)
                qk_half(wq_sb, qTa, 1, ih)
                qk_half(wk_sb, kTa, 1, ih, on_scalar=True)

            exp_tiles = [None] * NH
            items0 = [lambda ic=ic: v_chunk(ic) for ic in range(NT)]
            for dc in range(2, 4):
                items0.append(lambda dc=dc: qk_chunk(wq_sb, qTa, dc))
                items0.append(lambda dc=dc: qk_chunk(wk_sb, kTa, dc))
            exp_tiles[0] = s_head(0, items0)

            for h in range(1, NH):
                items = do_items(h - 1, exp_tiles[h - 1])
                if h + 1 < NH:
                    for dc in range(2 * (h + 1), 2 * (h + 1) + 2):
                        items.append(lambda dc=dc: qk_chunk(wq_sb, qTa, dc))
                        items.append(lambda dc=dc: qk_chunk(wk_sb, kTa, dc))
                exp_tiles[h] = s_head(h, items)

            # ---- tail: last head's denom/O interleaved with the out-proj ------
            rbc3 = expp.tile([P, N_TOK], F32, tag="rbc")
            expSt3 = exp_tiles[NH - 1]
            for ih in range(2):
                den_chunk(NH - 1, expSt3, rbc3, ih)
            o_chunk(NH - 1, expSt3, rbc3, 0, 0)
            o_chunk(NH - 1, expSt3, rbc3, 1, 0)
            for it in range(4):
                ph3_tile(it)
                if it < 2:
                    o_chunk(NH - 1, expSt3, rbc3, it, 1)
            for it in range(4, NT):
                ph3_tile(it)

    nc.compile()
    return nc


_NC_CACHE = {}


def _get_program(with_bias):
    key = ("nc", with_bias)
    if key not in _NC_CACHE:
        _NC_CACHE[key] = _build_program(with_bias)
    return _NC_CACHE[key]


def _fp8(a):
    return np.clip(a, -240.0, 240.0).astype(ml_dtypes.float8_e4m3fn)


def _make_in_maps(x, W_qkv, W_out, b_out):
    B = x.shape[0]
    x = np.ascontiguousarray(x.reshape(B, N_TOK, C), dtype=np.float32).astype(
        ml_dtypes.bfloat16
    )
    # W_qkv [C, h*3C]: head-major columns; q slot < C, k slot < 2C, v rest.
    w = np.asarray(W_qkv, dtype=np.float32).reshape(C, NH, 3 * C)
    wq = _fp8(np.ascontiguousarray(w[:, :, :C].reshape(C, NH * HD)) * 16.0)
    wk = _fp8(np.ascontiguousarray(w[:, :, C:2 * C].reshape(C, NH * HD)) * 16.0)
    wv = _fp8(np.ascontiguousarray(w[:, :, 2 * C:].reshape(C, NH * HD)) * 16.0)
    wo = _fp8(np.asarray(W_out, dtype=np.float32) * 4.0)
    bo = np.ascontiguousarray(
        np.asarray(b_out, dtype=np.float32).reshape(1, C) * 32.0
    )
    return [
        {"x": x[b], "wq": wq, "wk": wk, "wv": wv, "wo": wo, "bo": bo}
        for b in range(B)
    ]


def run_spmd(x, W_qkv, W_out, b_out, **runner_kwargs):
    """Run on the 8 cores; returns (BassKernelResults, assembled output)."""
    with_bias = bool(np.any(np.asarray(b_out)))
    nc = _get_program(with_bias)
    in_maps = _make_in_maps(x, W_qkv, W_out, b_out)
    res = bass_utils.run_bass_kernel_spmd(
        nc, in_maps, core_ids=list(range(8)), **runner_kwargs
    )
    B, H, W = x.shape[0], x.shape[1], x.shape[2]
    y = np.stack([res.results[b]["y"] for b in range(B)])
    return res, y.reshape(B, H, W, C).astype(np.float32)


def kernel(x, W_qkv, W_out, b_out):
    _, y = run_spmd(x, W_qkv, W_out, b_out)
    return y
